# revision 1
# baseline (speedup 1.0000x reference)
"""Bass/Trainium2 kernel for DoubleGRUModel (ragged double GRU + head).

8-way data-parallel (8 items/core). Software-pipelined: layer-2 trails
layer-1 by one 16-step block; their per-step tails hide under each
other's matmuls. Per For_i iteration (2 blocks):
  gi1(2b) batch; then 16x [L1 step(2b,j); L2 step(2b-1,j)]; gi2(2b);
  gi1(2b+1);    then 16x [L1 step(2b+1,j); L2 step(2b,j)]; gi2(2b+1)
L2's virtual block -1 runs against an all-dead gi (z-gate +30) so it
leaves h2 frozen at its init value. Ragged freezing likewise folds
30*dead(t,i) into the z-gate at gi-batch time; no per-step mask ops.
"""

import numpy as np

B, T, I, H = 64, 2048, 256, 512
NCORES = 8
IB = B // NCORES          # items per core = 8
TB = 16                   # time steps per block (=> 128-row gi tiles)

_compiled = None


def _build(t_steps=T, dbg=False):
    import concourse.bacc as bacc
    import concourse.bass as bass
    import concourse.mybir as mybir
    import concourse.tile as tile
    from contextlib import ExitStack

    fp16 = mybir.dt.float16
    fp32 = mybir.dt.float32
    AF = mybir.ActivationFunctionType
    OP = mybir.AluOpType

    nblk = t_steps // TB
    assert nblk % 2 == 0
    rows = IB * t_steps          # t-major rows: row = t*IB + i

    nc = bacc.Bacc("TRN2", target_bir_lowering=False, debug=False,
                   num_devices=NCORES)

    def din(name, shape, dt=fp16):
        return nc.dram_tensor(name, list(shape), dt, kind="ExternalInput").ap()

    xT = din("xT", [2, 128, rows])            # x.T k-chunks, t-major rows
    wi1 = din("wi1", [128, 2, 3, 512])        # Wi1.T  [p, k, gate, hcol]
    wh1 = din("wh1", [128, 4, 3, 512])
    wi2 = din("wi2", [128, 4, 3, 512])
    wh2 = din("wh2", [128, 4, 3, 512])
    bias1 = din("bias1", [1, 1536])           # bi1 (+bh1 for r,z)
    bias2 = din("bias2", [1, 1536])
    bh1n = din("bh1n", [1, 512])              # bh1 n-slice
    bh2n = din("bh2n", [1, 512])
    deadz = din("deadz", [1, rows])           # 30.0 where t >= len_i else 0
    onesr = din("onesr", [1, 512])            # ones row
    ones1 = din("ones1", [1, 128])
    selt = din("selt", [128, TB, IB])         # selector: [j*IB+i, j, i] = 1
    ident = din("ident", [IB, IB])
    gib0 = din("gib0", [128, 3, 512])         # zeros with z-gate = +30
    h1r0 = din("h1r0", [IB, 512])
    h2r0 = din("h2r0", [IB, 512])
    h1t0 = din("h1t0", [128, 4, IB])
    h2t0 = din("h2t0", [128, 4, IB])
    w1t = din("w1t", [128, 4, 128])           # W1.T
    b1r = din("b1r", [1, 128])
    w2rep = din("w2rep", [IB, 128], fp32)
    b2rep = din("b2rep", [IB, 1], fp32)
    y = nc.dram_tensor("y", [IB, 1], fp32, kind="ExternalOutput").ap()

    EPE = mybir.EngineType

    with tile.TileContext(nc) as tc, ExitStack() as top:
        singles = top.enter_context(tc.tile_pool(name="singles", bufs=1))

        def load_single(src, shape, name, dt=fp16):
            t_ = singles.tile(list(shape), dt, tag=name, name=name)
            nc.sync.dma_start(out=t_, in_=src)
            return t_

        wi1_sb = load_single(wi1, [128, 2, 3, 512], "wi1")
        wh1_sb = load_single(wh1, [128, 4, 3, 512], "wh1")
        wi2_sb = load_single(wi2, [128, 4, 3, 512], "wi2")
        wh2_sb = load_single(wh2, [128, 4, 3, 512], "wh2")
        bias1_sb = load_single(bias1, [1, 1536], "bias1")
        bias2_sb = load_single(bias2, [1, 1536], "bias2")
        bh1n_sb = load_single(bh1n, [1, 512], "bh1n")
        bh2n_sb = load_single(bh2n, [1, 512], "bh2n")
        onesr_sb = load_single(onesr, [1, 512], "onesr")
        ones1_sb = load_single(ones1, [1, 128], "ones1")
        selt_sb = load_single(selt, [128, TB, IB], "selt")
        ident_sb = load_single(ident, [IB, IB], "ident")
        w1t_sb = load_single(w1t, [128, 4, 128], "w1t")
        b1r_sb = load_single(b1r, [1, 128], "b1r")
        w2_sb = load_single(w2rep, [IB, 128], "w2rep", fp32)
        b2_sb = load_single(b2rep, [IB, 1], "b2rep", fp32)

        # persistent state
        hrow1 = [singles.tile([IB, 512], fp16, tag=f"hrow1_{j}",
                              name=f"hrow1_{j}") for j in range(2)]
        hrow2 = [singles.tile([IB, 512], fp16, tag=f"hrow2_{j}",
                              name=f"hrow2_{j}") for j in range(2)]
        ring1 = singles.tile([128, 4, TB, IB], fp16, tag="ring1", name="ring1")
        ring2 = singles.tile([128, 4, TB, IB], fp16, tag="ring2", name="ring2")
        # giB ping-pong (layer-2 gi, lives across iteration boundary)
        giB = [singles.tile([128, 3, 512], fp16, tag=f"giB{j}",
                            name=f"giB{j}") for j in range(2)]
        giNB = [singles.tile([IB, TB, 512], fp16, tag=f"giNB{j}",
                             name=f"giNB{j}") for j in range(2)]
        nc.sync.dma_start(out=hrow1[0], in_=h1r0)
        nc.sync.dma_start(out=hrow2[0], in_=h2r0)
        nc.sync.dma_start(out=ring1[:, :, TB - 1, :], in_=h1t0)
        nc.sync.dma_start(out=ring2[:, :, TB - 1, :], in_=h2t0)
        nc.sync.dma_start(out=giB[0], in_=gib0)
        for jj in range(TB):
            nc.sync.dma_start(out=giNB[0][:, jj, :],
                              in_=gib0[0:IB, 2, :])

        with ExitStack() as ctx:
            xp = ctx.enter_context(tc.tile_pool(name="xp", bufs=3))
            gp = ctx.enter_context(tc.tile_pool(name="gp", bufs=2))
            pg = ctx.enter_context(tc.tile_pool(name="pg", bufs=1, space="PSUM"))
            pp = ctx.enter_context(tc.tile_pool(name="pp", bufs=1, space="PSUM"))
            pt = ctx.enter_context(tc.tile_pool(name="pt", bufs=1, space="PSUM"))
            tp = ctx.enter_context(tc.tile_pool(name="tp", bufs=4))

            def gi_tile(k_chunks, lhs_tiles, w_sb, bias_sb, dead_blk,
                        out_sb, outn_sb):
                """Batched gi for one block into out_sb [128,3,512] fp16;
                n-gate also scattered per-step to outn_sb [IB,TB,512]."""
                for g in range(3):
                    psG = pg.tile([128, 512], fp32, tag="psG", name="psG")
                    for k in range(k_chunks):
                        nc.tensor.matmul(psG, lhs_tiles[k],
                                         w_sb[:, k, g, :],
                                         start=(k == 0), stop=False)
                    nc.tensor.matmul(psG, ones1_sb[0:1, :],
                                     bias_sb[0:1, 512 * g:512 * (g + 1)],
                                     start=False, stop=(g != 1))
                    if g == 1:  # z gate: add 30*dead
                        nc.tensor.matmul(psG, dead_blk,
                                         onesr_sb, start=False, stop=True)
                    nc.scalar.copy(out_sb[:, g, :], psG)
                    if g == 2:
                        for jj in range(TB):
                            nc.sync.dma_start(
                                out=outn_sb[:, jj, :],
                                in_=out_sb[IB * jj:IB * (jj + 1), 2, :])

            def step_mm(j, gi_sb, wh_sb, bhn_sb, ring, layer):
                jp = (j - 1) % TB
                ps = pp.tile([IB, 3, 512], fp32, tag=f"ps{layer}",
                             name=f"ps{layer}")
                for g, slot in ((0, 0), (2, 1), (1, 2)):  # r, n, z banks
                    for k in range(4):
                        nc.tensor.matmul(ps[:, slot, :],
                                         ring[:, k, jp, :],
                                         wh_sb[:, k, g, :],
                                         start=(k == 0), stop=False)
                    if g < 2:
                        nc.tensor.matmul(ps[:, slot, :],
                                         selt_sb[:, j, :],
                                         gi_sb[:, g, :],
                                         start=False, stop=True)
                    else:
                        nc.tensor.matmul(ps[:, slot, :],
                                         ones1_sb[0:1, 0:IB],
                                         bhn_sb, start=False, stop=True)
                return ps

            def tail_pre(j, ps, giN_sb, hrow, layer):
                cur = j % 2

                def tl(nm):
                    return tp.tile([IB, 512], fp16, tag=f"{nm}{layer}",
                                   name=f"{nm}{layer}")

                rb = tl("rb")
                nc.scalar.activation(rb, ps[:, 0, :], AF.Sigmoid)
                zb = tl("zb")
                nc.scalar.activation(zb, ps[:, 2, :], AF.Sigmoid)
                zc = tl("zc")
                nc.vector.tensor_scalar(zc, zb, -1.0, 1.0,
                                        op0=OP.mult, op1=OP.add)
                t1 = tl("t1")
                nc.vector.tensor_mul(t1, rb, ps[:, 1, :])
                zh = tl("zh")
                nc.vector.tensor_mul(zh, zb, hrow[cur])
                an = tl("an")
                nc.vector.tensor_tensor(an, t1, giN_sb[:, j, :], OP.add)
                return an, zc, zh

            def tail_post(j, pre, hrow, ring, layer):
                nxt = (j + 1) % 2
                an, zc, zh = pre

                def tl(nm):
                    return tp.tile([IB, 512], fp16, tag=f"{nm}{layer}",
                                   name=f"{nm}{layer}")

                nb = tl("nb")
                nc.scalar.activation(nb, an, AF.Tanh)
                zn = tl("zn")
                nc.vector.tensor_mul(zn, zc, nb)
                nc.vector.tensor_add(hrow[nxt], zn, zh)
                pst = pt.tile([128, 4, IB], fp16, tag="pst", name="pst")
                for c in range(4):
                    nc.tensor.transpose(pst[:, c, :],
                                        hrow[nxt][:, 128 * c:128 * (c + 1)],
                                        ident_sb)
                nc.vector.tensor_copy(ring[:, :, j, :], pst)



            def half(b_rt, parity):
                """L1 block (b) + interleaved L2 block (b-1).
                parity: compile-time 0/1 = b%2 (b_rt = 2*it + parity)."""
                lhsX = xp.tile([128, 2, 128], fp16, tag="lhsX", name="lhsX")
                for k in range(2):
                    nc.sync.dma_start(out=lhsX[:, k, :],
                                      in_=xT[k, :, bass.ts(b_rt, 128)])
                dead_blk = xp.tile([1, 128], fp16, tag="dead_blk",
                                   name="dead_blk")
                nc.sync.dma_start(out=dead_blk,
                                  in_=deadz[0:1, bass.ts(b_rt, 128)])
                giA = gp.tile([128, 3, 512], fp16, tag="giA", name="giA")
                giNA = gp.tile([IB, TB, 512], fp16, tag="giNA", name="giNA")
                gi_tile(2, [lhsX[:, 0, :], lhsX[:, 1, :]], wi1_sb, bias1_sb,
                        dead_blk, giA, giNA)
                pre2c = None
                for j in range(TB):
                    psA = step_mm(j, giA, wh1_sb, bh1n_sb, ring1, 1)
                    if pre2c is not None:
                        tail_post(j - 1, pre2c, hrow2, ring2, 2)
                    preA = tail_pre(j, psA, giNA, hrow1, 1)
                    psB = step_mm(j, giB[parity], wh2_sb, bh2n_sb, ring2, 2)
                    tail_post(j, preA, hrow1, ring1, 1)
                    pre2c = tail_pre(j, psB, giNB[parity], hrow2, 2)
                tail_post(TB - 1, pre2c, hrow2, ring2, 2)
                # gi2 for block b (consumed by L2 next half)
                gi_tile(4, [ring1[:, k, :, :] for k in range(4)],
                        wi2_sb, bias2_sb, dead_blk, giB[1 - parity],
                        giNB[1 - parity])

            with tc.For_i(0, nblk // 2, 1,
                          hint_engines=(EPE.PE, EPE.DVE, EPE.Activation,
                                        EPE.SP)) as it:
                half(it * 2, 0)
                half(it * 2 + 1, 1)

            # epilogue: final L2 block (nblk-1), uses giB[0]; L1 runs a
            # harmless extra block against the all-dead gi (state frozen,
            # ring1 garbage is never read afterwards)
            for j in range(TB):
                ps = step_mm(j, giB[0], wh2_sb, bh2n_sb, ring2, 2)
                pre = tail_pre(j, ps, giNB[0], hrow2, 2)
                tail_post(j, pre, hrow2, ring2, 2)

        # ===================== head =====================
        with ExitStack() as ctx:
            ep = ctx.enter_context(tc.tile_pool(name="ep", bufs=1))
            epp = ctx.enter_context(tc.tile_pool(name="epp", bufs=1, space="PSUM"))
            ps1 = epp.tile([IB, 128], fp32)
            for k in range(4):
                nc.tensor.matmul(ps1, ring2[:, k, TB - 1, :], w1t_sb[:, k, :],
                                 start=(k == 0), stop=False)
            nc.tensor.matmul(ps1, ones1_sb[0:1, 0:IB], b1r_sb[0:1, :],
                             start=False, stop=True)
            o1 = ep.tile([IB, 128], fp32)
            nc.vector.tensor_copy(o1, ps1)
            t2 = ep.tile([IB, 128], fp32)
            nc.vector.tensor_mul(t2, o1, w2_sb)
            red = ep.tile([IB, 1], fp32)
            nc.vector.tensor_reduce(red, t2, axis=mybir.AxisListType.X, op=OP.add)
            out_t = ep.tile([IB, 1], fp32)
            nc.vector.tensor_scalar(out_t, red, b2_sb[:, 0:1], None, op0=OP.add)
            nc.sync.dma_start(out=y, in_=out_t)

    nc.compile()
    return nc


def _prep_core(c, x, lengths, state0, state1, Wi1, Wh1, bi1, bh1,
               Wi2, Wh2, bi2, bh2, W1, b1, W2, b2, t_steps=T):
    f16 = np.float16
    rows = IB * t_steps
    xs = np.asarray(x)[c * IB:(c + 1) * IB, :t_steps]        # [IB, T, I]
    ls = np.asarray(lengths)[c * IB:(c + 1) * IB]            # [IB]
    # t-major rows: row = t*IB + i
    xT = np.ascontiguousarray(
        xs.transpose(1, 0, 2).reshape(rows, I).T.astype(f16)
    ).reshape(2, 128, rows)

    def wstack(WT, kc):       # [K, 1536] -> [128, kc, 3, 512]
        w = WT.astype(f16).reshape(kc, 128, 3, 512)
        return np.ascontiguousarray(w.transpose(1, 0, 2, 3))

    Wi1T = np.asarray(Wi1).T.astype(np.float32)              # [256, 1536]
    Wh1T = np.asarray(Wh1).T.astype(np.float32)              # [512, 1536]
    Wi2T = np.asarray(Wi2).T.astype(np.float32)
    Wh2T = np.asarray(Wh2).T.astype(np.float32)
    bi1_, bh1_ = np.asarray(bi1, np.float32), np.asarray(bh1, np.float32)
    bi2_, bh2_ = np.asarray(bi2, np.float32), np.asarray(bh2, np.float32)
    bias1 = bi1_.copy(); bias1[:1024] += bh1_[:1024]
    bias2 = bi2_.copy(); bias2[:1024] += bh2_[:1024]

    tgrid = np.arange(t_steps)
    dead = (tgrid[:, None] >= ls[None, :]).astype(np.float32)  # [T, IB]
    deadz = (30.0 * dead).reshape(1, rows).astype(f16)

    selt = np.zeros((128, TB, IB), f16)
    for j in range(TB):
        for i in range(IB):
            selt[j * IB + i, j, i] = 1.0

    gib0 = np.zeros((128, 3, 512), f16)
    gib0[:, 1, :] = 30.0

    s0 = np.broadcast_to(np.asarray(state0, np.float32), (IB, H)).astype(f16)
    s1 = np.broadcast_to(np.asarray(state1, np.float32), (IB, H)).astype(f16)
    s0t = np.ascontiguousarray(s0.T.reshape(4, 128, IB).transpose(1, 0, 2))
    s1t = np.ascontiguousarray(s1.T.reshape(4, 128, IB).transpose(1, 0, 2))

    W1T = np.asarray(W1).T.astype(f16)                       # [512, 128]
    return {
        "xT": xT,
        "wi1": wstack(Wi1T, 2), "wh1": wstack(Wh1T, 4),
        "wi2": wstack(Wi2T, 4), "wh2": wstack(Wh2T, 4),
        "bias1": bias1.astype(f16)[None, :],
        "bias2": bias2.astype(f16)[None, :],
        "bh1n": bh1_[1024:].astype(f16)[None, :],
        "bh2n": bh2_[1024:].astype(f16)[None, :],
        "deadz": deadz,
        "onesr": np.ones((1, 512), f16),
        "ones1": np.ones((1, 128), f16),
        "selt": selt,
        "ident": np.eye(IB, dtype=f16),
        "gib0": gib0,
        "h1r0": s0, "h2r0": s1, "h1t0": s0t, "h2t0": s1t,
        "w1t": np.ascontiguousarray(
            W1T.reshape(4, 128, 128).transpose(1, 0, 2)),
        "b1r": np.asarray(b1, f16)[None, :],
        "w2rep": np.broadcast_to(np.asarray(W2, np.float32)[0], (IB, 128)).copy(),
        "b2rep": np.full((IB, 1), float(np.asarray(b2).reshape(-1)[0]), np.float32),
    }


def kernel(**inputs):
    global _compiled
    from concourse.bass_utils import run_bass_kernel_spmd
    if _compiled is None:
        _compiled = _build()
    in_maps = [_prep_core(c, **inputs) for c in range(NCORES)]
    res = run_bass_kernel_spmd(_compiled, in_maps, core_ids=list(range(NCORES)))
    out = np.concatenate([res.results[c]["y"] for c in range(NCORES)], axis=0)
    return out.astype(np.float32)

